# revision 6
# baseline (speedup 1.0000x reference)
"""Trainium2 Bass kernel for CnnKF observation-IR contraction.

Computes out[b, o] = sum_{i, l} observation_IR[b, i, l, o] * context[b, R-1-l, i]
for B=2048, R=32, O=64, data-parallel over 8 NeuronCores.

Per system b the contraction is a matvec: with k = i*R + l,
    A_b = observation_IR[b] viewed as [K=2048, O=64]   (contiguous 512KB in DRAM)
    v_b[k] = context[b, R-1-(k%R), k//R]
    out[b] = A_b^T v_b

Per-core layout (256 systems/core):
  A_b is reshaped [128, 16*64]: partition p holds rows k = 16p..16p+15 (4KB
  contiguous per partition -> ideal DMA).  The contraction runs as 16
  PSUM-accumulated matmuls (sub = 0..15), each contracting k = 16p+sub over
  the 128 partitions.  To batch G=8 systems per matmul, the stationary
  operand is [128, 8] of context values (column g = v_{b0+g}[16p+sub]) and
  the moving operand is [128, 8*64] of IR slices; the useful results are
  the 8 diagonal [1, 64] blocks of the [8, 512] PSUM tile (the off-diagonal
  7/8 of the MACs are discarded - the PE has ~10x compute headroom over the
  HBM stream here).  float32r runs the PE at 1 cycle/row.

  Compute engines can only address SBUF windows starting at partition
  0/32/64/96, so the diagonal cannot be gathered with per-partition
  copies.  Instead: multiply the PSUM tile by the constant mask
  I_8 (x) ones(64) (zeroing the off-diagonal blocks), then contract the 8
  partitions with a ones-vector matmul, which packs the 8 useful [1,64]
  blocks into a single [1, 512] PSUM row.
"""

import os
import numpy as np

B, R, O = 2048, 32, 64
NCORES = 8
BP = B // NCORES        # 256 systems per core
K = R * O               # 2048 contraction length
P = 128                 # SBUF partitions
SUB = K // P            # 16 k-subchunks per partition
G = 8                   # systems per matmul group (N = G*O = 512)
NG = BP // G            # 32 groups per core

USE_F32R = True

_CACHE = {}


def _build_program():
    from concourse import bacc, tile, mybir

    nc = bacc.Bacc("TRN2", target_bir_lowering=False, debug=False,
                   num_devices=NCORES)
    in_dt = mybir.dt.float32r if USE_F32R else mybir.dt.float32
    ir = nc.dram_tensor("ir", [BP, P, SUB * O], in_dt,
                        kind="ExternalInput").ap()
    vt = nc.dram_tensor("vt", [P, SUB, BP], in_dt,
                        kind="ExternalInput").ap()
    mask = nc.dram_tensor("mask", [G, G * O], mybir.dt.float32,
                          kind="ExternalInput").ap()
    out = nc.dram_tensor("out", [NG, G * O], mybir.dt.float32,
                         kind="ExternalOutput").ap()

    with tile.TileContext(nc) as tc:
        with (
            tc.tile_pool(name="const", bufs=1) as cpool,
            tc.tile_pool(name="acts", bufs=2) as apool,
            tc.tile_pool(name="work", bufs=3) as wpool,
            tc.tile_pool(name="psum", bufs=4, space="PSUM") as ppool,
            tc.tile_pool(name="psum2", bufs=2, space="PSUM") as ppool2,
            tc.tile_pool(name="outp", bufs=1) as opool,
        ):
            vt_sb = cpool.tile([P, SUB, BP], in_dt)
            nc.sync.dma_start(out=vt_sb[:], in_=vt[:])
            mask_sb = cpool.tile([G, G * O], mybir.dt.float32)
            nc.sync.dma_start(out=mask_sb[:], in_=mask[:])
            ones_sb = cpool.tile([G, 1], mybir.dt.float32)
            nc.vector.memset(ones_sb[:], 1.0)
            out_sb = opool.tile([1, NG, G * O], mybir.dt.float32)

            for q in range(NG):
                t = apool.tile([P, G, SUB * O], in_dt)
                nc.sync.dma_start(
                    out=t[:],
                    in_=ir[q * G:(q + 1) * G].rearrange("g p c -> p g c"),
                )
                ps = ppool.tile([G, G * O], mybir.dt.float32)
                for sub in range(SUB):
                    lhsT = vt_sb[:, sub, q * G:(q + 1) * G]
                    rhs = t[:, :, sub * O:(sub + 1) * O]
                    nc.tensor.matmul(ps[:], lhsT, rhs,
                                     start=(sub == 0), stop=(sub == SUB - 1))
                # zero the off-diagonal blocks, then pack the diagonal into
                # one [1, 512] row by contracting partitions with ones
                mprod = wpool.tile([G, G * O], mybir.dt.float32)
                nc.vector.tensor_mul(mprod[:], ps[:], mask_sb[:])
                ps2 = ppool2.tile([1, G * O], mybir.dt.float32)
                nc.tensor.matmul(ps2[:], ones_sb[:], mprod[:],
                                 start=True, stop=True)
                nc.vector.tensor_copy(out_sb[0:1, q, :], ps2[0:1, :])

            nc.sync.dma_start(out=out.rearrange("q n -> (q n)").unsqueeze(0),
                              in_=out_sb[:])

    nc.compile()
    return nc


def _get_program():
    if "nc" not in _CACHE:
        _CACHE["nc"] = _build_program()
    return _CACHE["nc"]


_MASK = np.kron(np.eye(G, dtype=np.float32),
                np.ones((1, O), dtype=np.float32)).reshape(G, G * O)


def _prep_core_inputs(context, observation_IR, core):
    b0 = core * BP
    # zero-copy view: [BP, O, R, O] -> [BP, K, O] -> [BP, P, SUB*O]
    ir = observation_IR[b0:b0 + BP].reshape(BP, P, SUB * O)
    # v_all[b, k] = context[b, R-1-(k%R), k//R]  (flip time, transpose)
    ctx = context[b0:b0 + BP]
    v_all = np.ascontiguousarray(ctx[:, ::-1, :].transpose(0, 2, 1)).reshape(BP, K)
    # vt[p, sub, b] = v_all[b, 16p+sub]
    vt = np.ascontiguousarray(v_all.reshape(BP, P, SUB).transpose(1, 2, 0))
    return {"ir": np.ascontiguousarray(ir), "vt": vt, "mask": _MASK}


def run(context, observation_IR, trace=False):
    from concourse.bass_utils import run_bass_kernel_spmd

    context = np.asarray(context, dtype=np.float32)
    observation_IR = np.asarray(observation_IR, dtype=np.float32)
    nc = _get_program()
    in_maps = [_prep_core_inputs(context, observation_IR, c)
               for c in range(NCORES)]
    res = run_bass_kernel_spmd(nc, in_maps, core_ids=list(range(NCORES)),
                               trace=trace)
    _CACHE["last_results"] = res
    # out[q, (g, o)] holds system b0 + q*G + g
    full = np.empty((B, O), dtype=np.float32)
    for c in range(NCORES):
        full[c * BP:(c + 1) * BP] = res.results[c]["out"].reshape(BP, O)
    return full


def kernel(**inputs):
    return run(inputs["context"], inputs["observation_IR"],
               trace=bool(int(os.environ.get("KERNEL_TRACE", "0"))))
